# revision 8
# baseline (speedup 1.0000x reference)
"""Trainium2 Bass kernel: batched 1-D linear interpolation on a uniform grid.

out[b, j] = (1 - w_j) * y[b, i_j] + w_j * y[b, i_j + 1]

Reformulated as a matmul  out = y @ G  with G[i_j, j] = 1 - w_j and
G[i_j + 1, j] = w_j (2 nonzeros per column, known on the host from x_new).
Queries are sorted by bin index i_j so that each chunk of <=128 sorted
queries touches a window of <=128 consecutive grid points; the chunk is then
a single 128x128x128 TensorE matmul:

    psum[rows128, q128] = yT_window[pts128, rows128].T @ G_chunk[pts128, q128]

y is shipped pre-transposed (points-major) and cast to bf16 on the host, so
each window is one contiguous 512 KiB DMA.  PSUM results are cast to bf16 on
the way to SBUF (DVE/ACT alternating) and streamed out in 1 MiB DMAs.  The
host un-permutes the sorted output columns during the unshard.

Sharding: pure data parallel over the batch axis across 8 NeuronCores
(y_points rows 16384 -> 8 x 2048); x_new-derived constants are replicated.
"""

import numpy as np

BATCH = 16384
NUM_POINTS = 2048
M = 4096
N_CORES = 8
ROWS_PER_CORE = BATCH // N_CORES  # 2048
P = 128
N_TILES = ROWS_PER_CORE // P  # 16
CH = 128  # queries per chunk (= G columns per matmul)

_NC_CACHE = {}
_PLAN_CACHE = {}


def _host_precompute(x_new):
    """Replicate the reference's searchsorted/weight math with the same jax
    ops on the same backend, so boundary decisions match the reference."""
    import jax.numpy as jnp

    x_new_j = jnp.asarray(np.asarray(x_new, dtype=np.float32))
    x_points = jnp.linspace(0.0, 1.0, NUM_POINTS, dtype=x_new_j.dtype)
    idxs = jnp.searchsorted(x_points, x_new_j, side="right") - 1
    idxs = jnp.clip(idxs, 0, NUM_POINTS - 2)
    x1 = x_points[idxs]
    x2 = x_points[idxs + 1]
    w = (x_new_j - x1) / (x2 - x1)
    return np.asarray(idxs).astype(np.int64), np.asarray(w, dtype=np.float32)


def _make_plan(x_new):
    """Sort queries by bin index and chunk greedily: each chunk holds up to
    CH sorted queries whose grid window [i_min, i_max+1] fits in 128 points.
    Returns (chunks, order) where chunks = [(p0, qlo, qhi)] over sorted
    positions and order = argsort of the queries."""
    idxs, w = _host_precompute(x_new)
    order = np.argsort(idxs, kind="stable")
    si = idxs[order]

    chunks = []
    qlo = 0
    while qlo < M:
        qhi = min(qlo + CH, M)
        # shrink until window fits: need points [i_min, i_max + 1], 128 wide
        while si[qhi - 1] - si[qlo] > P - 2:
            qhi -= 1
        p0 = int(min(si[qlo], NUM_POINTS - P))
        span = int(si[qhi - 1]) + 2 - p0  # points [p0, i_max+1] inclusive
        chunks.append((p0, qlo, qhi, span))
        qlo = qhi
    return chunks, order, idxs, w


def _build_nc(chunks):
    import concourse.bacc as bacc
    import concourse.mybir as mybir
    from concourse.tile import TileContext

    f32 = mybir.dt.float32
    bf16 = mybir.dt.bfloat16
    nch = len(chunks)

    nc = bacc.Bacc()
    yt = nc.dram_tensor("yt", [NUM_POINTS, ROWS_PER_CORE], bf16, kind="ExternalInput")
    g = nc.dram_tensor("g", [P, nch * CH], bf16, kind="ExternalInput")
    out = nc.dram_tensor("out", [ROWS_PER_CORE, nch * CH], bf16, kind="ExternalOutput")

    BANK = 512  # one full PSUM bank (fp32); 4 chunk-matmuls per bank
    MM_PER_BANK = BANK // CH
    nbank = (nch + MM_PER_BANK - 1) // MM_PER_BANK
    HALF = (nbank // 2) * BANK  # output strip width (bf16 cols)

    with TileContext(nc) as tc:
        with (
            tc.tile_pool(name="const", bufs=1) as cp,
            tc.tile_pool(name="psum", bufs=8, space="PSUM") as pp,
            tc.tile_pool(name="outp", bufs=4) as op,
        ):
            g_t = cp.tile([P, nch * CH], bf16, tag="g")
            win = [
                cp.tile([P, ROWS_PER_CORE], bf16, tag=f"win{c}", name=f"win{c}")
                for c in range(nch)
            ]
            # first matmul needs win0 + g cols 0:512 — issue those first,
            # split g so the first slice lands quickly.  Only the populated
            # span of each window is loaded (the rest of G is zero).
            p0, _, _, span = chunks[0]
            nc.sync.dma_start(out=win[0][:span, :], in_=yt[p0 : p0 + span, :])
            gq = (nch * CH) // 4
            for s in range(4):
                nc.sync.dma_start(
                    out=g_t[:, s * gq : (s + 1) * gq], in_=g[:, s * gq : (s + 1) * gq]
                )
            for c in range(1, nch):
                p0, _, _, span = chunks[c]
                nc.sync.dma_start(out=win[c][:span, :], in_=yt[p0 : p0 + span, :])

            k = 0
            for r in range(N_TILES):
                o_half = [
                    op.tile([P, HALF], bf16, tag="o0", name="o0"),
                    op.tile([P, nch * CH - HALF], bf16, tag="o1", name="o1"),
                ]
                for b in range(nbank):
                    clo = b * MM_PER_BANK
                    chi = min(clo + MM_PER_BANK, nch)
                    ps = pp.tile([P, BANK], f32, tag="ps")
                    for c in range(clo, chi):
                        span = chunks[c][3]
                        nc.tensor.matmul(
                            ps[:, (c - clo) * CH : (c - clo + 1) * CH],
                            win[c][:span, r * P : (r + 1) * P],
                            g_t[:span, c * CH : (c + 1) * CH],
                            start=True,
                            stop=True,
                        )
                    h = 1 if clo * CH >= HALF else 0
                    off = clo * CH - h * HALF
                    dst = o_half[h][:, off : off + (chi - clo) * CH]
                    src = ps[:, : (chi - clo) * CH]
                    # split the PSUM->SBUF cast copies across DVE and ACT (5:3)
                    if k % 8 < 5:
                        nc.vector.tensor_copy(out=dst, in_=src)
                    else:
                        nc.scalar.copy(dst, src)
                    k += 1
                    # flush each half-row strip as soon as its banks are
                    # copied; out-DMAs ride the second HWDGE ring (ACT) so
                    # they overlap the window loads on the sync ring
                    if chi * CH == HALF or chi == nch:
                        rows = slice(r * P, (r + 1) * P)
                        cols = slice(h * HALF, h * HALF + o_half[h].shape[-1])
                        nc.scalar.dma_start(out=out[rows, cols], in_=o_half[h][:])

    nc.compile()
    return nc


def _get_plan_and_nc(x_new):
    import ml_dtypes

    key = np.asarray(x_new, dtype=np.float32).tobytes()
    if key not in _PLAN_CACHE:
        chunks, order, idxs, w = _make_plan(x_new)
        nch = len(chunks)
        # G: [128 pts-in-window, nch*CH sorted queries], bf16
        gmat = np.zeros((P, nch * CH), dtype=np.float32)
        si = idxs[order]
        sw = w[order]
        for c, (p0, qlo, qhi, _) in enumerate(chunks):
            cols = c * CH + np.arange(qhi - qlo)
            gmat[si[qlo:qhi] - p0, cols] = 1.0 - sw[qlo:qhi]
            gmat[si[qlo:qhi] + 1 - p0, cols] = sw[qlo:qhi]
        gmat = gmat.astype(ml_dtypes.bfloat16)
        # natural output column for each device column slot (-1 = padding)
        cols_nat = np.full(nch * CH, -1, dtype=np.int64)
        for c, (p0, qlo, qhi, _) in enumerate(chunks):
            cols_nat[c * CH : c * CH + (qhi - qlo)] = order[qlo:qhi]
        _PLAN_CACHE[key] = (chunks, gmat, cols_nat)
    chunks, gmat, cols_nat = _PLAN_CACHE[key]

    nc_key = (len(chunks), tuple((p0, sp) for p0, _, _, sp in chunks))
    if nc_key not in _NC_CACHE:
        _NC_CACHE[nc_key] = _build_nc(chunks)
    return chunks, gmat, cols_nat, _NC_CACHE[nc_key]


def run(y_points, x_new, trace=False, **spmd_kwargs):
    """Run the Bass kernel; returns (output, BassKernelResults)."""
    import ml_dtypes
    from concourse.bass_utils import run_bass_kernel_spmd

    chunks, gmat, cols_nat, nc = _get_plan_and_nc(x_new)

    y16 = np.asarray(y_points, dtype=np.float32).astype(ml_dtypes.bfloat16)
    in_maps = []
    for c in range(N_CORES):
        ytc = np.ascontiguousarray(y16[c * ROWS_PER_CORE : (c + 1) * ROWS_PER_CORE].T)
        in_maps.append({"yt": ytc, "g": gmat})

    res = run_bass_kernel_spmd(
        nc, in_maps, list(range(N_CORES)), trace=trace, **spmd_kwargs
    )

    valid = cols_nat >= 0
    dst_cols = cols_nat[valid]
    out_full = np.empty((BATCH, M), dtype=np.float32)
    for c in range(N_CORES):
        o = res.results[c]["out"]
        if not valid.all():
            o = o[:, valid]
        out_full[c * ROWS_PER_CORE : (c + 1) * ROWS_PER_CORE, dst_cols] = o.astype(
            np.float32
        )
    return out_full, res


def kernel(y_points, x_new):
    out, _ = run(y_points, x_new)
    return out


# revision 9
# speedup vs baseline: 2.5283x; 2.5283x over previous
"""Trainium2 Bass kernel: batched 1-D linear interpolation on a uniform grid.

out[b, j] = (1 - w_j) * y[b, i_j] + w_j * y[b, i_j + 1]

Reformulated as a matmul  out = y @ G  with G[i_j, j] = 1 - w_j and
G[i_j + 1, j] = w_j (2 nonzeros per column, known on the host from x_new).

Device layout: y is shipped pre-transposed (points-major) in bf16.  The grid
is covered by 17 fixed windows of 128 consecutive points starting every 127
points, so a query with i_j in [127k, 127k+126] finds both of its points in
window k.  Queries are sorted by i_j and grouped into 8 PSUM banks of 512
columns; each bank is computed by one matmul per window segment overlapping
it (~1.5 segments per bank):

    psum[rows128, seg] = win_k[pts128, rows128].T @ G[pts128, seg]

PSUM banks are cast to bf16 on the way to SBUF (DVE/ACT split) and streamed
out in 512 KiB half-row DMAs on the second HWDGE ring so input and output
transfers overlap.  The host un-permutes the sorted output columns during
the unshard.

Sharding: pure data parallel over the batch axis across 8 NeuronCores
(y_points rows 16384 -> 8 x 2048); x_new-derived constants are replicated.
"""

import numpy as np

BATCH = 16384
NUM_POINTS = 2048
M = 4096
N_CORES = 8
ROWS_PER_CORE = BATCH // N_CORES  # 2048
P = 128
N_TILES = ROWS_PER_CORE // P  # 16
WGRID = P - 1  # window stride: 127 points
NWIN = (NUM_POINTS - 2) // WGRID + 1  # 17
BANK = 512  # fp32 columns per PSUM bank
NBANK = M // BANK  # 8

_NC_CACHE = {}
_PLAN_CACHE = {}


def _host_precompute(x_new):
    """Replicate the reference's searchsorted/weight math with the same jax
    ops on the same backend, so boundary decisions match the reference."""
    import jax.numpy as jnp

    x_new_j = jnp.asarray(np.asarray(x_new, dtype=np.float32))
    x_points = jnp.linspace(0.0, 1.0, NUM_POINTS, dtype=x_new_j.dtype)
    idxs = jnp.searchsorted(x_points, x_new_j, side="right") - 1
    idxs = jnp.clip(idxs, 0, NUM_POINTS - 2)
    x1 = x_points[idxs]
    x2 = x_points[idxs + 1]
    w = (x_new_j - x1) / (x2 - x1)
    return np.asarray(idxs).astype(np.int64), np.asarray(w, dtype=np.float32)


def _win_p0(k):
    return min(WGRID * k, NUM_POINTS - P)


def _make_plan(x_new):
    """Sort queries by bin index; build per-bank window segments.
    Returns (segs, order, idxs, w) where segs[b] = [(k, s0, s1)] covering
    sorted positions [b*BANK, (b+1)*BANK) split at window changes."""
    idxs, w = _host_precompute(x_new)
    order = np.argsort(idxs, kind="stable")
    si = idxs[order]
    kq = si // WGRID  # window id per sorted query

    segs = [[] for _ in range(NBANK)]
    s = 0
    while s < M:
        b = s // BANK
        k = int(kq[s])
        e = s + 1
        lim = (b + 1) * BANK
        while e < lim and kq[e] == k:
            e += 1
        segs[b].append((k, s, int(e)))
        s = e
    return segs, order, idxs, w


def _build_nc(segs):
    import concourse.bacc as bacc
    import concourse.mybir as mybir
    from concourse.tile import TileContext

    f32 = mybir.dt.float32
    bf16 = mybir.dt.bfloat16

    nc = bacc.Bacc()
    yt = nc.dram_tensor("yt", [NUM_POINTS, ROWS_PER_CORE], bf16, kind="ExternalInput")
    g = nc.dram_tensor("g", [P, M], bf16, kind="ExternalInput")
    out = nc.dram_tensor("out", [ROWS_PER_CORE, M], bf16, kind="ExternalOutput")

    HALF = (NBANK // 2) * BANK  # output strip width (bf16 cols)

    with TileContext(nc) as tc:
        with (
            tc.tile_pool(name="const", bufs=1) as cp,
            tc.tile_pool(name="psum", bufs=8, space="PSUM") as pp,
            tc.tile_pool(name="outp", bufs=4) as op,
        ):
            g_t = cp.tile([P, M], bf16, tag="g")
            win = [
                cp.tile([P, ROWS_PER_CORE], bf16, tag=f"win{k}", name=f"win{k}")
                for k in range(NWIN)
            ]
            # first matmuls need win0 + the first slice of g
            p0 = _win_p0(0)
            nc.sync.dma_start(out=win[0][:], in_=yt[p0 : p0 + P, :])
            for s in range(4):
                gq = M // 4
                nc.sync.dma_start(
                    out=g_t[:, s * gq : (s + 1) * gq], in_=g[:, s * gq : (s + 1) * gq]
                )
            for k in range(1, NWIN):
                p0 = _win_p0(k)
                nc.sync.dma_start(out=win[k][:], in_=yt[p0 : p0 + P, :])

            cnt = 0
            for r in range(N_TILES):
                o_half = [
                    op.tile([P, HALF], bf16, tag="o0", name="o0"),
                    op.tile([P, M - HALF], bf16, tag="o1", name="o1"),
                ]
                for b in range(NBANK):
                    ps = pp.tile([P, BANK], f32, tag="ps")
                    for k, s0, s1 in segs[b]:
                        nc.tensor.matmul(
                            ps[:, s0 - b * BANK : s1 - b * BANK],
                            win[k][:, r * P : (r + 1) * P],
                            g_t[:, s0:s1],
                            start=True,
                            stop=True,
                        )
                    h = 1 if b * BANK >= HALF else 0
                    dst = o_half[h][:, b * BANK - h * HALF : (b + 1) * BANK - h * HALF]
                    # split the PSUM->SBUF cast copies across DVE and ACT
                    # (~8:5 — ACT also pays the out-DMA issue cost)
                    if cnt % 13 < 8:
                        nc.vector.tensor_copy(out=dst, in_=ps[:])
                    else:
                        nc.scalar.copy(dst, ps[:])
                    cnt += 1
                    # flush each half-row strip as soon as its banks are
                    # copied; out-DMAs ride the second HWDGE ring (ACT) so
                    # they overlap the window loads on the sync ring
                    if (b + 1) * BANK in (HALF, M):
                        rows = slice(r * P, (r + 1) * P)
                        cols = slice(h * HALF, h * HALF + o_half[h].shape[-1])
                        nc.scalar.dma_start(out=out[rows, cols], in_=o_half[h][:])

    nc.compile()
    return nc


def _get_plan_and_nc(x_new):
    import ml_dtypes

    key = np.asarray(x_new, dtype=np.float32).tobytes()
    if key not in _PLAN_CACHE:
        segs, order, idxs, w = _make_plan(x_new)
        # G: [128 pts-in-window, M sorted queries], bf16
        si = idxs[order]
        sw = w[order]
        p0s = np.array([_win_p0(k) for k in range(NWIN)])
        off = p0s[si // WGRID]
        gmat = np.zeros((P, M), dtype=np.float32)
        cols = np.arange(M)
        gmat[si - off, cols] = 1.0 - sw
        gmat[si + 1 - off, cols] = sw
        gmat = gmat.astype(ml_dtypes.bfloat16)
        _PLAN_CACHE[key] = (segs, gmat, order)
    segs, gmat, order = _PLAN_CACHE[key]

    nc_key = tuple(tuple(s) for b in segs for s in b)
    if nc_key not in _NC_CACHE:
        _NC_CACHE[nc_key] = _build_nc(segs)
    return segs, gmat, order, _NC_CACHE[nc_key]


def run(y_points, x_new, trace=False, **spmd_kwargs):
    """Run the Bass kernel; returns (output, BassKernelResults)."""
    import ml_dtypes
    from concourse.bass_utils import run_bass_kernel_spmd

    segs, gmat, order, nc = _get_plan_and_nc(x_new)

    y16 = np.asarray(y_points, dtype=np.float32).astype(ml_dtypes.bfloat16)
    in_maps = []
    for c in range(N_CORES):
        ytc = np.ascontiguousarray(y16[c * ROWS_PER_CORE : (c + 1) * ROWS_PER_CORE].T)
        in_maps.append({"yt": ytc, "g": gmat})

    res = run_bass_kernel_spmd(
        nc, in_maps, list(range(N_CORES)), trace=trace, **spmd_kwargs
    )

    out_full = np.empty((BATCH, M), dtype=np.float32)
    for c in range(N_CORES):
        o = res.results[c]["out"]
        out_full[c * ROWS_PER_CORE : (c + 1) * ROWS_PER_CORE, order] = o.astype(
            np.float32
        )
    return out_full, res


def kernel(y_points, x_new):
    out, _ = run(y_points, x_new)
    return out
